# revision 38
# baseline (speedup 1.0000x reference)
"""TRN2 Bass kernel for nn_Attention_87497073754296.

Computes, for Y [4096, 1024] f32 and W_param [1024, 1024] f32:
    G = Y @ W_param.T ; S = G @ G.T ; A = softmax(S, -1) ; Z = A @ Y
using the identity S = Y @ (W_param.T @ W_param) @ Y.T, so each core only
needs its own row-shard of the queries plus the (replicated) full Y — no
collectives. M = W_param.T @ W_param (symmetric, d x d) is computed once
on the host and replicated.

Sharding: rows of Y (queries), 512 per core across 8 cores. The key axis
is rotated per core (softmax is permutation-invariant) so the core's own
512 query columns are yt block 0, which doubles as H's moving operand.

Per core:
    Ht  = (Yq @ M).T        fp8e4m3 DoubleRow matmuls
    S   = Ht.T @ Y.T        fp8e4m3 DoubleRow; evicted fp16, chunk maxes
                            taken straight from PSUM on DVE in parallel
    P   = exp(S16 - max16)  ACT exp (both halves queued up front),
                            accum_out row sums
    Pt8 = fp8(P).T          PE transposes -> PSUM -> ACT/DVE copy-cast
    Z   = (Pt8.T @ (Yh8 ; Ym8)) * (1/rowsum)
where Yh8 = fp8(Y), Ym8 = fp8(Y - Yh8); both A@Y passes run as fp8
DoubleRow matmuls accumulating into the same PSUM banks, carrying ~8
mantissa bits (~0.1%% relative error) at the fp8 157 TF/s rate.

Numerical invariant: the chunk maxes are stored fp16. Rounding the fp32
PSUM max to fp16 equals the max of the fp16-evicted S (rounding is
monotone), so exp(S16 - max) is exactly 1.0 on the argmax element and
the fp8 quantization of P cancels against the fp32 rowsum.

Throughput notes baked into the structure:
  - every accumulation round-robins >=4 PSUM regions (consecutive
    matmuls into the same region pay a ~46 ns RMW turnaround);
  - yt is packed column-block-major and loaded in 0.5 MB blocks so
    scoring starts as blocks land; Yh8/Ym8 are fully resident before
    the A@Y phase (no mid-phase DMA stalls);
  - A@Y runs two accumulators at a time; each pair evicts + stores
    while the next pair accumulates; softmax(3) weaves into the first
    pair's matmuls;
  - a warmup matmul train ramps the PE clock while the first loads
    land (the HAM gate needs ~3 us of continuous PE activity).
"""
import numpy as np
import ml_dtypes

import concourse.bass as bass
import concourse.mybir as mybir
import concourse.tile as tile
from concourse import bacc
from concourse.bass_utils import run_bass_kernel_spmd
from concourse.masks import make_identity
F32 = mybir.dt.float32
FP16 = mybir.dt.float16
FP8 = mybir.dt.float8e4
DR = mybir.MatmulPerfMode.DoubleRow
AF = mybir.ActivationFunctionType
AX = mybir.AxisListType
OP = mybir.AluOpType

N, D = 4096, 1024
CORES = 8
QSH = N // CORES          # 512 queries per core
P = 128                   # partitions
DT = D // P               # 8 d-subtiles
QT = QSH // P             # 4 q-tiles per core
JC = N // 512             # 8 j-chunks of 512 for scores
JT = N // P               # 32 j-tiles of 128 for A@Y
KP = JT // 2              # 16 DoubleRow j-tile pairs for A@Y
KEEP = 4                  # Y8 pairs retained for the 4th-accumulator catch-up
WARMUP = 106

_CACHED = {}


def _build():
    nc = bacc.Bacc("TRN2", target_bir_lowering=False, debug=False,
                   num_devices=CORES)
    # packed [P, sub * free] layouts: each partition reads one contiguous run
    M8 = nc.declare_dram_parameter("M8", [P, DT * D], FP8, isOutput=False)
    # Yt8 packed column-block-major and per-core rotated so the core's own
    # 512 query columns are block 0 (it doubles as H's moving operand)
    Yt8 = nc.declare_dram_parameter("Yt8", [P, JC * DT * 512], FP8,
                                    isOutput=False)
    Yh8 = nc.declare_dram_parameter("Yh8", [P, JT * D], FP8, isOutput=False)
    Ym8 = nc.declare_dram_parameter("Ym8", [P, JT * D], FP8, isOutput=False)
    Z = nc.declare_dram_parameter("Z", [QSH, D], F32, isOutput=True)

    with tile.TileContext(nc) as tc:
        with (
            tc.tile_pool(name="const", bufs=1) as const,
            tc.tile_pool(name="stat", bufs=1) as stat,
            tc.tile_pool(name="htpool", bufs=1) as htpool,
            tc.tile_pool(name="spool", bufs=1) as spool,
            tc.tile_pool(name="epool", bufs=3) as epool,
            tc.tile_pool(name="ptpool", bufs=1) as ptpool,
            tc.tile_pool(name="mpool", bufs=1) as mpool,
            tc.tile_pool(name="ytpool", bufs=1) as ytpool,
            tc.tile_pool(name="yzpool", bufs=1) as yzpool,
            tc.tile_pool(name="zopool", bufs=2) as zopool,
        ):
            # warmup tile initialized on DVE (no gpsimd ucode-load delay);
            # plain matmuls (not transposes) so any simulator run is legal
            wtile = const.tile([P, P], FP16, name="wtile")
            nc.vector.memset(wtile[:], 1.0)
            with tc.tile_pool(name="warm", bufs=1, space="PSUM") as warm:
                wp = warm.tile([P, P], F32, name="wp")
                for _ in range(WARMUP):
                    nc.tensor.matmul(wp[:], wtile[:], wtile[:],
                                     start=True, stop=True)

            # ---- resident loads (host-packed, contiguous per partition;
            # yt split in halves so scoring can start on the first half) ----
            m_sb = mpool.tile([P, DT, D], FP8, name="m_sb")
            yt_blks = [
                ytpool.tile([P, DT, 512], FP8, name=f"yt_blk{i}",
                            tag=f"yt{i}")
                for i in range(JC)
            ]
            yh_sb = yzpool.tile([P, JT, D], FP8, name="yh_sb", tag="yh")
            ym_sb = yzpool.tile([P, JT, D], FP8, name="ym_sb", tag="ym")
            # block 0 + M first (H's operands), then the score key blocks
            # in consumption order, then the A@Y operands
            nc.gpsimd.dma_start(yt_blks[0][:], Yt8[:, :DT * 512])
            nc.sync.dma_start(m_sb[:], M8[:, :])
            for i in range(1, JC):
                q = nc.sync if i % 2 else nc.gpsimd
                q.dma_start(yt_blks[i][:],
                            Yt8[:, i * DT * 512:(i + 1) * DT * 512])
            nc.sync.dma_start(yh_sb[:], Yh8[:, :])
            nc.gpsimd.dma_start(ym_sb[:], Ym8[:, :])

            ident = const.tile([P, P], FP16, name="ident")
            make_identity(nc, ident[:])

            # stats: negmax/recip [P, QT], half sums, rowsum
            st = stat.tile([P, 64], F32, name="st")
            negmax = st[:, 0:QT]
            recip = st[:, QT:2 * QT]
            rowsum = st[:, 40:44]
            esum = st[:, 44:44 + 2 * QT]
            # chunk maxes in fp16: rounding the fp32 PSUM max to fp16
            # equals the max of the fp16-evicted S (rounding is monotone),
            # so exp(S16 - max) hits exactly 1.0 on the argmax element and
            # the fp8 quantization of P cancels against the rowsum
            st16 = stat.tile([P, 2 * QT * JC], FP16, name="st16")
            mx8 = st16[:, :QT * JC]
            mx4 = st16[:, QT * JC:QT * JC + 4 * QT]

            ht_sb = [
                htpool.tile([P, 2, QSH], FP8, name=f"ht_sb{c}", tag=f"ht{c}")
                for c in range(DT // 2)
            ]
            ht_pairs = [
                [ht_sb[s][:, :, t * P:(t + 1) * P] for t in range(QT)]
                for s in range(DT // 2)
            ]
            s_sb = [
                spool.tile([P, N], FP16, name=f"s_sb{t}", tag=f"s{t}")
                for t in range(QT)
            ]
            pt_sb = [
                ptpool.tile([P, N], FP8, name=f"pt_sb{t}", tag=f"pt{t}")
                for t in range(QT)
            ]
            e_tiles = {}

            def emit_exp_half(t, h, pool):
                e = pool.tile([P, N // 2], FP16, name="e_sb", tag="e")
                e_tiles[(t, h)] = e
                nc.scalar.activation(
                    e[:], s_sb[t][:, h * (N // 2):(h + 1) * (N // 2)],
                    AF.Exp, bias=negmax[:, t:t + 1], scale=1.0,
                    accum_out=esum[:, 2 * t + h:2 * t + h + 1],
                )

            def emit_T_chunk2(t, c2, pool):
                """Transpose 1024 cols (two 512-chunks) of P(t) into
                pt_sb[t]: 8 PE transposes into one 1-bank PSUM tile, one
                wide copy-cast out (gpsimd cannot read PSUM, so alternate
                ACT and DVE)."""
                e = e_tiles[(t, c2 // 2)]
                off = (c2 % 2) * 1024
                pp = pool.tile([P, 1024], FP16, name="pp", tag="pt", bufs=2)
                for k in range(8):
                    nc.tensor.transpose(
                        pp[:, k * P:(k + 1) * P],
                        e[:, off + k * P: off + (k + 1) * P],
                        ident[:],
                    )
                dst = pt_sb[t][:, c2 * 1024:(c2 + 1) * 1024]
                if c2 % 2 == 1:
                    nc.vector.tensor_copy(dst, pp[:])
                else:
                    nc.scalar.copy(dst, pp[:])

            def emit_finish_rowsum(t):
                nc.vector.tensor_reduce(
                    rowsum[:, t:t + 1], esum[:, 2 * t:2 * t + 2],
                    axis=AX.X, op=OP.add,
                )
                nc.vector.reciprocal(recip[:, t:t + 1], rowsum[:, t:t + 1])

            def emit_negmax(t):
                nc.vector.tensor_reduce(
                    negmax[:, t:t + 1], mx4[:, 4 * t:4 * t + 4],
                    axis=AX.X, op=OP.max, negate=True,
                )

            # ---- H: Ht[do, q] = sum_di M[di,do].T @ Yqt[di, q]; all 8
            # accumulators round-robin so same-region PSUM writes (RMW
            # turnaround) are 8 matmuls apart and nothing waits on evicts
            with tc.tile_pool(name="psH", bufs=1, space="PSUM") as psH:
                hps = [psH.tile([P, QSH], F32, name=f"hp{c}", tag=f"h{c}")
                       for c in range(DT)]
                for s in range(DT // 2):
                    for c in range(DT):
                        nc.tensor.matmul(
                            hps[c][:],
                            m_sb[:, 2 * s:2 * s + 2, c * P:(c + 1) * P],
                            yt_blks[0][:, 2 * s:2 * s + 2, :],
                            start=(s == 0), stop=(s == DT // 2 - 1),
                            perf_mode=DR,
                        )
                        if s == DT // 2 - 1:
                            # evict as soon as each accumulator stops — all
                            # on DVE, which is otherwise idle until S(0)'s
                            # maxes; keeps ACT free to start S(0) evictions
                            nc.vector.tensor_copy(
                                ht_sb[c // 2][:, c % 2, :], hps[c][:])

            with tc.tile_pool(name="ps", bufs=4, space="PSUM") as ps:

                # ---- S (t-outer), softmax of t-1 interleaved per group;
                # 4 chunks accumulate round-robin so same-region PSUM
                # writes are 4 matmuls apart ----
                for t in range(QT):
                    if t >= 1:
                        # half 0 + negmax were already queued inside tile
                        # t-1's own group 1 (they only need g0's evictions
                        # and the maxes); half 1 needs g1's evictions, so
                        # it goes here — still ahead of this tile's ACT work
                        emit_exp_half(t - 1, 1, epool)
                    for g in range(2):
                        sps = [ps.tile([P, 1024], F32, name="sp", tag="s",
                                       bufs=3)
                               for _ in range(2)]
                        for s in range(DT // 2):
                            lhs = ht_pairs[s][t]
                            for c in range(4):
                                nc.tensor.matmul(
                                    sps[c // 2][:, (c % 2) * 512:
                                                (c % 2 + 1) * 512],
                                    lhs,
                                    yt_blks[4 * g + c][:, 2 * s:2 * s + 2, :],
                                    start=(s == 0), stop=(s == DT // 2 - 1),
                                    perf_mode=DR,
                                )
                        # 1024-wide chunk maxes straight from PSUM on DVE,
                        # queued before any copies: they free the sp ring
                        # and feed negmax (latency-critical DVE work)
                        for c2 in range(2):
                            j2 = 2 * g + c2
                            nc.vector.tensor_reduce(
                                mx4[:, 4 * t + j2: 4 * t + j2 + 1],
                                sps[c2][:], axis=AX.X, op=OP.max,
                            )
                        if g == 1:
                            # all 4 chunk maxes of this tile are queued:
                            # negmax + exp half 0 jump ahead of the g1
                            # evictions so the next tile's transposes
                            # never wait on the exp
                            emit_negmax(t)
                            emit_exp_half(t, 0, epool)
                        for c2 in range(2):
                            j2 = 2 * g + c2
                            dst = s_sb[t][:, j2 * 1024:(j2 + 1) * 1024]
                            if (g, c2) == (1, 0):
                                nc.vector.tensor_copy(dst, sps[c2][:])
                            else:
                                nc.scalar.copy(dst, sps[c2][:])
                        if t >= 1:
                            for c2 in range(2):
                                emit_T_chunk2(t - 1, 2 * g + c2, ps)
                    if t >= 1:
                        emit_finish_rowsum(t - 1)

            # ---- A@Y, one accumulator at a time; softmax(3) woven into
            # the first accumulator's matmuls; each tile evicts + stores
            # while the next accumulates ----
            tl = QT - 1
            with tc.tile_pool(name="psZ", bufs=2, space="PSUM") as psZ:
                emit_exp_half(tl, 1, epool)
                for tp in range(QT // 2):
                    ts = (2 * tp, 2 * tp + 1)
                    zps = {t: psZ.tile([P, D], F32, name=f"zp{t}", tag="z",
                                       bufs=3)
                           for t in ts}
                    for k in range(KP):
                        lhss = {
                            t: pt_sb[t][:, k * 256:(k + 1) * 256].rearrange(
                                "p (a b) -> p a b", a=2)
                            for t in ts
                        }
                        # 4 PSUM regions round-robin (2 tiles x 2 halves)
                        for pi, ysrc in ((0, yh_sb), (1, ym_sb)):
                            for t in ts:
                                for dc in range(2):
                                    nc.tensor.matmul(
                                        zps[t][:, dc * 512:(dc + 1) * 512],
                                        lhss[t],
                                        ysrc[:, 2 * k:2 * k + 2,
                                             dc * 512:dc * 512 + 512],
                                        start=(k == 0 and pi == 0),
                                        stop=(k == KP - 1 and pi == 1),
                                        perf_mode=DR,
                                    )
                        if tp == 0:
                            if k < 4:
                                emit_T_chunk2(tl, k, psZ)
                            if k == 3:
                                emit_finish_rowsum(tl)
                    for t in ts:
                        # evict + store overlap the next accumulator pair
                        zo = zopool.tile([P, D], F32, name="zo", tag="zo")
                        nc.scalar.activation(
                            zo[:, :512], zps[t][:, :512], AF.Copy, bias=0.0,
                            scale=recip[:, t:t + 1],
                        )
                        nc.vector.tensor_scalar_mul(
                            zo[:, 512:], zps[t][:, 512:], recip[:, t:t + 1])
                        nc.sync.dma_start(
                            Z[t * P:(t + 1) * P, :512], zo[:, :512])
                        nc.sync.dma_start(
                            Z[t * P:(t + 1) * P, 512:], zo[:, 512:])

    nc.finalize()
    return nc


def _pack(x8: np.ndarray) -> np.ndarray:
    """[sub*P, F] -> [P, sub*F]: partition-contiguous subtile-major."""
    sp, f = x8.shape
    sub = sp // P
    return np.ascontiguousarray(
        x8.reshape(sub, P, f).transpose(1, 0, 2).reshape(P, sub * f))


def _prep_inputs(Y: np.ndarray, W_param: np.ndarray):
    Y = np.ascontiguousarray(Y, dtype=np.float32)
    W = np.ascontiguousarray(W_param, dtype=np.float32)
    M = (W.T @ W).astype(np.float32)
    e4 = ml_dtypes.float8_e4m3
    Yh8 = Y.astype(e4)
    Ym8 = (Y - Yh8.astype(np.float32)).astype(e4)
    M8p = _pack(M.astype(e4))
    Yt8 = np.ascontiguousarray(Y.T).astype(e4)
    # [dt, p, jc-block, col] view for the column-block-major yt pack
    Yt8v = Yt8.reshape(DT, P, JC, 512)
    in_maps = []
    for c in range(CORES):
        # per-core rotation of the key axis: block 0 = this core's own
        # queries (doubles as H's moving operand); softmax is invariant to
        # the key permutation and the A@Y operands are rolled to match
        rot = np.roll(Yt8v, -c, axis=2)
        yt_p = np.ascontiguousarray(
            rot.transpose(1, 2, 0, 3)).reshape(P, JC * DT * 512)
        yh_p = _pack(np.roll(Yh8, -c * QSH, axis=0))
        ym_p = _pack(np.roll(Ym8, -c * QSH, axis=0))
        in_maps.append({
            "M8": M8p,
            "Yt8": yt_p,
            "Yh8": yh_p,
            "Ym8": ym_p,
        })
    return in_maps


def _run(inputs: dict, trace: bool = False):
    Y = np.asarray(inputs["Y"])
    W = np.asarray(inputs["W_param"])
    assert Y.shape == (N, D) and W.shape == (D, D)
    if "nc" not in _CACHED:
        _CACHED["nc"] = _build()
    nc = _CACHED["nc"]
    in_maps = _prep_inputs(Y, W)
    res = run_bass_kernel_spmd(nc, in_maps, list(range(CORES)), trace=trace)
    out = np.concatenate(
        [res.results[c]["Z"] for c in range(CORES)], axis=0
    ).astype(np.float32)
    return out, res


def kernel(Y: np.ndarray, W_param: np.ndarray) -> np.ndarray:
    out, _ = _run({"Y": Y, "W_param": W_param})
    return out


# revision 39
# speedup vs baseline: 1.0429x; 1.0429x over previous
"""TRN2 Bass kernel for nn_Attention_87497073754296.

Computes, for Y [4096, 1024] f32 and W_param [1024, 1024] f32:
    G = Y @ W_param.T ; S = G @ G.T ; A = softmax(S, -1) ; Z = A @ Y
using the identity S = Y @ (W_param.T @ W_param) @ Y.T, so each core only
needs its own row-shard of the queries plus the (replicated) full Y — no
collectives. M = W_param.T @ W_param (symmetric, d x d) is computed once
on the host and replicated.

Sharding: rows of Y (queries), 512 per core across 8 cores. The key axis
is rotated per core (softmax is permutation-invariant) so the core's own
512 query columns are yt block 0, which doubles as H's moving operand.

Per core:
    Ht  = (Yq @ M).T        fp8e4m3 DoubleRow matmuls
    S   = Ht.T @ Y.T        fp8e4m3 DoubleRow; evicted fp16, chunk maxes
                            taken straight from PSUM on DVE in parallel
    P   = exp(S16 - max16)  ACT exp (both halves queued up front),
                            accum_out row sums
    Pt8 = fp8(P).T          PE transposes -> PSUM -> ACT/DVE copy-cast
    Z   = (Pt8.T @ (Yh8 ; Ym8)) * (1/rowsum)
where Yh8 = fp8(Y), Ym8 = fp8(Y - Yh8); both A@Y passes run as fp8
DoubleRow matmuls accumulating into the same PSUM banks, carrying ~8
mantissa bits (~0.1%% relative error) at the fp8 157 TF/s rate.

Numerical invariant: the chunk maxes are stored fp16. Rounding the fp32
PSUM max to fp16 equals the max of the fp16-evicted S (rounding is
monotone), so exp(S16 - max) is exactly 1.0 on the argmax element and
the fp8 quantization of P cancels against the fp32 rowsum.

Throughput notes baked into the structure:
  - every accumulation round-robins >=4 PSUM regions (consecutive
    matmuls into the same region pay a ~46 ns RMW turnaround);
  - yt is packed column-block-major and loaded in 0.5 MB blocks so
    scoring starts as blocks land; Yh8/Ym8 are fully resident before
    the A@Y phase (no mid-phase DMA stalls);
  - A@Y runs two accumulators at a time; each pair evicts + stores
    while the next pair accumulates; softmax(3) weaves into the first
    pair's matmuls;
  - a warmup matmul train ramps the PE clock while the first loads
    land (the HAM gate needs ~3 us of continuous PE activity).
"""
import numpy as np
import ml_dtypes

import concourse.bass as bass
import concourse.mybir as mybir
import concourse.tile as tile
from concourse import bacc
from concourse.bass_utils import run_bass_kernel_spmd
from concourse.masks import make_identity
F32 = mybir.dt.float32
FP16 = mybir.dt.float16
FP8 = mybir.dt.float8e4
DR = mybir.MatmulPerfMode.DoubleRow
AF = mybir.ActivationFunctionType
AX = mybir.AxisListType
OP = mybir.AluOpType

N, D = 4096, 1024
CORES = 8
QSH = N // CORES          # 512 queries per core
P = 128                   # partitions
DT = D // P               # 8 d-subtiles
QT = QSH // P             # 4 q-tiles per core
JC = N // 512             # 8 j-chunks of 512 for scores
JT = N // P               # 32 j-tiles of 128 for A@Y
KP = JT // 2              # 16 DoubleRow j-tile pairs for A@Y
KEEP = 4                  # Y8 pairs retained for the 4th-accumulator catch-up
WARMUP = 106

_CACHED = {}


def _build():
    nc = bacc.Bacc("TRN2", target_bir_lowering=False, debug=False,
                   num_devices=CORES)
    # packed [P, sub * free] layouts: each partition reads one contiguous run
    M8 = nc.declare_dram_parameter("M8", [P, DT * D], FP8, isOutput=False)
    # Yt8 packed column-block-major and per-core rotated so the core's own
    # 512 query columns are block 0 (it doubles as H's moving operand)
    Yt8 = nc.declare_dram_parameter("Yt8", [P, JC * DT * 512], FP8,
                                    isOutput=False)
    Yh8 = nc.declare_dram_parameter("Yh8", [P, JT * D], FP8, isOutput=False)
    Ym8 = nc.declare_dram_parameter("Ym8", [P, JT * D], FP8, isOutput=False)
    Z = nc.declare_dram_parameter("Z", [QSH, D], F32, isOutput=True)

    with tile.TileContext(nc) as tc:
        with (
            tc.tile_pool(name="const", bufs=1) as const,
            tc.tile_pool(name="stat", bufs=1) as stat,
            tc.tile_pool(name="htpool", bufs=1) as htpool,
            tc.tile_pool(name="spool", bufs=1) as spool,
            tc.tile_pool(name="epool", bufs=3) as epool,
            tc.tile_pool(name="ptpool", bufs=1) as ptpool,
            tc.tile_pool(name="mpool", bufs=1) as mpool,
            tc.tile_pool(name="ytpool", bufs=1) as ytpool,
            tc.tile_pool(name="yzpool", bufs=1) as yzpool,
            tc.tile_pool(name="zopool", bufs=2) as zopool,
        ):
            # warmup tile initialized on DVE (no gpsimd ucode-load delay);
            # plain matmuls (not transposes) so any simulator run is legal
            wtile = const.tile([P, P], FP16, name="wtile")
            nc.vector.memset(wtile[:], 1.0)
            with tc.tile_pool(name="warm", bufs=1, space="PSUM") as warm:
                wp = warm.tile([P, P], F32, name="wp")
                for _ in range(WARMUP):
                    nc.tensor.matmul(wp[:], wtile[:], wtile[:],
                                     start=True, stop=True)

            # ---- resident loads (host-packed, contiguous per partition;
            # yt split in halves so scoring can start on the first half) ----
            m_sb = mpool.tile([P, DT, D], FP8, name="m_sb")
            yt_blks = [
                ytpool.tile([P, DT, 512], FP8, name=f"yt_blk{i}",
                            tag=f"yt{i}")
                for i in range(JC)
            ]
            yh_sb = yzpool.tile([P, JT, D], FP8, name="yh_sb", tag="yh")
            ym_sb = yzpool.tile([P, JT, D], FP8, name="ym_sb", tag="ym")
            # block 0 + M first (H's operands), then the score key blocks
            # in consumption order, then the A@Y operands
            nc.gpsimd.dma_start(yt_blks[0][:], Yt8[:, :DT * 512])
            nc.sync.dma_start(m_sb[:], M8[:, :])
            for i in range(1, JC):
                q = nc.sync if i % 2 else nc.gpsimd
                q.dma_start(yt_blks[i][:],
                            Yt8[:, i * DT * 512:(i + 1) * DT * 512])
            nc.sync.dma_start(yh_sb[:], Yh8[:, :])
            nc.gpsimd.dma_start(ym_sb[:], Ym8[:, :])

            ident = const.tile([P, P], FP16, name="ident")
            make_identity(nc, ident[:])

            # stats: negmax/recip [P, QT], half sums, rowsum
            st = stat.tile([P, 64], F32, name="st")
            negmax = st[:, 0:QT]
            recip = st[:, QT:2 * QT]
            rowsum = st[:, 40:44]
            esum = st[:, 44:44 + 2 * QT]
            # chunk maxes in fp16: rounding the fp32 PSUM max to fp16
            # equals the max of the fp16-evicted S (rounding is monotone),
            # so exp(S16 - max) hits exactly 1.0 on the argmax element and
            # the fp8 quantization of P cancels against the rowsum
            st16 = stat.tile([P, 2 * QT * JC], FP16, name="st16")
            mx8 = st16[:, :QT * JC]
            mx4 = st16[:, QT * JC:QT * JC + 4 * QT]

            ht_sb = [
                htpool.tile([P, 2, QSH], FP8, name=f"ht_sb{c}", tag=f"ht{c}")
                for c in range(DT // 2)
            ]
            ht_pairs = [
                [ht_sb[s][:, :, t * P:(t + 1) * P] for t in range(QT)]
                for s in range(DT // 2)
            ]
            s_sb = [
                spool.tile([P, N], FP16, name=f"s_sb{t}", tag=f"s{t}")
                for t in range(QT)
            ]
            pt_sb = [
                ptpool.tile([P, N], FP8, name=f"pt_sb{t}", tag=f"pt{t}")
                for t in range(QT)
            ]
            e_tiles = {}

            def emit_exp_half(t, h, pool):
                e = pool.tile([P, N // 2], FP16, name="e_sb", tag="e")
                e_tiles[(t, h)] = e
                nc.scalar.activation(
                    e[:], s_sb[t][:, h * (N // 2):(h + 1) * (N // 2)],
                    AF.Exp, bias=negmax[:, t:t + 1], scale=1.0,
                    accum_out=esum[:, 2 * t + h:2 * t + h + 1],
                )

            def emit_T_chunk2(t, c2, pool):
                """Transpose 1024 cols (two 512-chunks) of P(t) into
                pt_sb[t]: 8 PE transposes into one 1-bank PSUM tile, one
                wide copy-cast out (gpsimd cannot read PSUM, so alternate
                ACT and DVE)."""
                e = e_tiles[(t, c2 // 2)]
                off = (c2 % 2) * 1024
                pp = pool.tile([P, 1024], FP16, name="pp", tag="pt", bufs=2)
                for k in range(8):
                    nc.tensor.transpose(
                        pp[:, k * P:(k + 1) * P],
                        e[:, off + k * P: off + (k + 1) * P],
                        ident[:],
                    )
                dst = pt_sb[t][:, c2 * 1024:(c2 + 1) * 1024]
                if c2 % 2 == 1:
                    nc.vector.tensor_copy(dst, pp[:])
                else:
                    nc.scalar.copy(dst, pp[:])

            def emit_finish_rowsum(t):
                nc.vector.tensor_reduce(
                    rowsum[:, t:t + 1], esum[:, 2 * t:2 * t + 2],
                    axis=AX.X, op=OP.add,
                )
                nc.vector.reciprocal(recip[:, t:t + 1], rowsum[:, t:t + 1])

            def emit_negmax(t):
                nc.vector.tensor_reduce(
                    negmax[:, t:t + 1], mx4[:, 4 * t:4 * t + 4],
                    axis=AX.X, op=OP.max, negate=True,
                )

            # ---- H: Ht[do, q] = sum_di M[di,do].T @ Yqt[di, q]; all 8
            # accumulators round-robin so same-region PSUM writes (RMW
            # turnaround) are 8 matmuls apart and nothing waits on evicts
            with tc.tile_pool(name="psH", bufs=1, space="PSUM") as psH:
                hps = [psH.tile([P, QSH], F32, name=f"hp{c}", tag=f"h{c}")
                       for c in range(DT)]
                for s in range(DT // 2):
                    for c in range(DT):
                        nc.tensor.matmul(
                            hps[c][:],
                            m_sb[:, 2 * s:2 * s + 2, c * P:(c + 1) * P],
                            yt_blks[0][:, 2 * s:2 * s + 2, :],
                            start=(s == 0), stop=(s == DT // 2 - 1),
                            perf_mode=DR,
                        )
                        if s == DT // 2 - 1:
                            # evict as soon as each accumulator stops,
                            # alternating engines; S's first matmuls only
                            # need ht pair 0 so the pipeline overlaps
                            dst = ht_sb[c // 2][:, c % 2, :]
                            if c % 2 == 0:
                                nc.scalar.copy(dst, hps[c][:])
                            else:
                                nc.vector.tensor_copy(dst, hps[c][:])

            with tc.tile_pool(name="ps", bufs=4, space="PSUM") as ps:

                # ---- S (t-outer), softmax of t-1 interleaved per group;
                # 4 chunks accumulate round-robin so same-region PSUM
                # writes are 4 matmuls apart ----
                for t in range(QT):
                    if t >= 1:
                        # half 0 + negmax were already queued inside tile
                        # t-1's own group 1 (they only need g0's evictions
                        # and the maxes); half 1 needs g1's evictions, so
                        # it goes here — still ahead of this tile's ACT work
                        emit_exp_half(t - 1, 1, epool)
                    for g in range(2):
                        sps = [ps.tile([P, 1024], F32, name="sp", tag="s",
                                       bufs=3)
                               for _ in range(2)]
                        for s in range(DT // 2):
                            lhs = ht_pairs[s][t]
                            for c in range(4):
                                nc.tensor.matmul(
                                    sps[c // 2][:, (c % 2) * 512:
                                                (c % 2 + 1) * 512],
                                    lhs,
                                    yt_blks[4 * g + c][:, 2 * s:2 * s + 2, :],
                                    start=(s == 0), stop=(s == DT // 2 - 1),
                                    perf_mode=DR,
                                )
                        # 1024-wide chunk maxes straight from PSUM on DVE,
                        # queued before any copies: they free the sp ring
                        # and feed negmax (latency-critical DVE work)
                        for c2 in range(2):
                            j2 = 2 * g + c2
                            nc.vector.tensor_reduce(
                                mx4[:, 4 * t + j2: 4 * t + j2 + 1],
                                sps[c2][:], axis=AX.X, op=OP.max,
                            )
                        if g == 1:
                            # all 4 chunk maxes of this tile are queued:
                            # negmax + exp half 0 jump ahead of the g1
                            # evictions so the next tile's transposes
                            # never wait on the exp
                            emit_negmax(t)
                            emit_exp_half(t, 0, epool)
                        for c2 in range(2):
                            j2 = 2 * g + c2
                            dst = s_sb[t][:, j2 * 1024:(j2 + 1) * 1024]
                            if (g, c2) == (1, 0):
                                nc.vector.tensor_copy(dst, sps[c2][:])
                            else:
                                nc.scalar.copy(dst, sps[c2][:])
                        if t >= 1:
                            for c2 in range(2):
                                emit_T_chunk2(t - 1, 2 * g + c2, ps)
                    if t >= 1:
                        emit_finish_rowsum(t - 1)

            # ---- A@Y, one accumulator at a time; softmax(3) woven into
            # the first accumulator's matmuls; each tile evicts + stores
            # while the next accumulates ----
            tl = QT - 1
            with tc.tile_pool(name="psZ", bufs=2, space="PSUM") as psZ:
                emit_exp_half(tl, 1, epool)
                for tp in range(QT // 2):
                    ts = (2 * tp, 2 * tp + 1)
                    zps = {t: psZ.tile([P, D], F32, name=f"zp{t}", tag="z",
                                       bufs=3)
                           for t in ts}
                    for k in range(KP):
                        lhss = {
                            t: pt_sb[t][:, k * 256:(k + 1) * 256].rearrange(
                                "p (a b) -> p a b", a=2)
                            for t in ts
                        }
                        # 4 PSUM regions round-robin (2 tiles x 2 halves)
                        for pi, ysrc in ((0, yh_sb), (1, ym_sb)):
                            for t in ts:
                                for dc in range(2):
                                    nc.tensor.matmul(
                                        zps[t][:, dc * 512:(dc + 1) * 512],
                                        lhss[t],
                                        ysrc[:, 2 * k:2 * k + 2,
                                             dc * 512:dc * 512 + 512],
                                        start=(k == 0 and pi == 0),
                                        stop=(k == KP - 1 and pi == 1),
                                        perf_mode=DR,
                                    )
                        if tp == 0:
                            if k < 4:
                                emit_T_chunk2(tl, k, psZ)
                            if k == 3:
                                emit_finish_rowsum(tl)
                    for t in ts:
                        # evict + store overlap the next accumulator pair
                        zo = zopool.tile([P, D], F32, name="zo", tag="zo")
                        nc.scalar.activation(
                            zo[:, :512], zps[t][:, :512], AF.Copy, bias=0.0,
                            scale=recip[:, t:t + 1],
                        )
                        nc.vector.tensor_scalar_mul(
                            zo[:, 512:], zps[t][:, 512:], recip[:, t:t + 1])
                        nc.sync.dma_start(
                            Z[t * P:(t + 1) * P, :512], zo[:, :512])
                        nc.sync.dma_start(
                            Z[t * P:(t + 1) * P, 512:], zo[:, 512:])

    nc.finalize()
    return nc


def _pack(x8: np.ndarray) -> np.ndarray:
    """[sub*P, F] -> [P, sub*F]: partition-contiguous subtile-major."""
    sp, f = x8.shape
    sub = sp // P
    return np.ascontiguousarray(
        x8.reshape(sub, P, f).transpose(1, 0, 2).reshape(P, sub * f))


def _prep_inputs(Y: np.ndarray, W_param: np.ndarray):
    Y = np.ascontiguousarray(Y, dtype=np.float32)
    W = np.ascontiguousarray(W_param, dtype=np.float32)
    M = (W.T @ W).astype(np.float32)
    e4 = ml_dtypes.float8_e4m3
    Yh8 = Y.astype(e4)
    Ym8 = (Y - Yh8.astype(np.float32)).astype(e4)
    M8p = _pack(M.astype(e4))
    Yt8 = np.ascontiguousarray(Y.T).astype(e4)
    # [dt, p, jc-block, col] view for the column-block-major yt pack
    Yt8v = Yt8.reshape(DT, P, JC, 512)
    in_maps = []
    for c in range(CORES):
        # per-core rotation of the key axis: block 0 = this core's own
        # queries (doubles as H's moving operand); softmax is invariant to
        # the key permutation and the A@Y operands are rolled to match
        rot = np.roll(Yt8v, -c, axis=2)
        yt_p = np.ascontiguousarray(
            rot.transpose(1, 2, 0, 3)).reshape(P, JC * DT * 512)
        yh_p = _pack(np.roll(Yh8, -c * QSH, axis=0))
        ym_p = _pack(np.roll(Ym8, -c * QSH, axis=0))
        in_maps.append({
            "M8": M8p,
            "Yt8": yt_p,
            "Yh8": yh_p,
            "Ym8": ym_p,
        })
    return in_maps


def _run(inputs: dict, trace: bool = False):
    Y = np.asarray(inputs["Y"])
    W = np.asarray(inputs["W_param"])
    assert Y.shape == (N, D) and W.shape == (D, D)
    if "nc" not in _CACHED:
        _CACHED["nc"] = _build()
    nc = _CACHED["nc"]
    in_maps = _prep_inputs(Y, W)
    res = run_bass_kernel_spmd(nc, in_maps, list(range(CORES)), trace=trace)
    out = np.concatenate(
        [res.results[c]["Z"] for c in range(CORES)], axis=0
    ).astype(np.float32)
    return out, res


def kernel(Y: np.ndarray, W_param: np.ndarray) -> np.ndarray:
    out, _ = _run({"Y": Y, "W_param": W_param})
    return out
